# revision 10
# baseline (speedup 1.0000x reference)
"""Trainium2 Bass kernel for nn_Net0 (20-layer width-7 MLP + log_softmax).

Strategy: 8-way data parallel over batch. Per core, 64 batch chunks are
packed 4-chunks-per-32-partition-band (7 features each) so every 7x7 layer
becomes 16 concurrent 32x32 PE-tile matmuls (exact fp32, ~77ns per 512-col
MM measured). Activations stay resident in SBUF; bias+ReLU evacuation is
split across ScalarE and VectorE. Final log_softmax collapses to
-softplus(+/-d) with d = (W21[1]-W21[0]).h + (b21[1]-b21[0]); the sign flip
happens on the host.

Band ping-pong: PE tile (r,c) reads SBUF partitions 32r..32r+27 and writes
PSUM partition quadrant c (bank r). The partition-preserving evacuation of
bank b writes free-slot b, so a chunk processed by tile (r,c) at layer l is
processed by tile (c,r) at layer l+1. With tiles instantiated for all 16
(r,c) pairs each layer, the code is identical every layer.
"""

import os
import numpy as np

B = 4194304
H = 7
N_MID = 19
N_CORES = 8
R_CORE = B // N_CORES          # 524288
NSTR = 16                      # stripes per chunk
SLOT = 512                     # columns per stripe-slot
CHUNK = NSTR * SLOT            # 8192 rows per chunk; 64 chunks per core
FREE = 4 * CHUNK               # act free size (4 slots of CHUNK, stripe-major)
N_LAYERS = 21                  # fc1 + 19 mid + fin

_CACHE = {}


def _pack_weights(W1, b1, Wmid, bmid, W21, b21):
    """Build w_all [128, 588] and bias_all [128, 21] host-side."""
    w_all = np.zeros((128, 32 * N_LAYERS), np.float32)
    bias_all = np.zeros((128, N_LAYERS), np.float32)
    wd = (W21[1] - W21[0]).astype(np.float32)        # [7]
    delta = np.float32(b21[1] - b21[0])
    for i in range(4):                                # band
        for bp in range(4):                           # block within band
            # fc1: K rows 32i+2bp+u, M cols 7bp+f  -> W1[f, u]
            for u in range(2):
                w_all[32 * i + 2 * bp + u, 7 * bp:7 * bp + 7] = W1[:, u]
            for l in range(N_MID):
                c0 = 32 * (l + 1)
                # rows 32i+7bp+fin, cols c0+7bp+fout -> Wmid[l, fout, fin]
                w_all[32 * i + 7 * bp:32 * i + 7 * bp + 7,
                      c0 + 7 * bp:c0 + 7 * bp + 7] = Wmid[l].T
            # fin: rows 32i+7bp+f, cols 560 + 2bp+sign -> +/- wd[f]
            c0 = 32 * 20
            w_all[32 * i + 7 * bp:32 * i + 7 * bp + 7, c0 + 2 * bp] = wd
            w_all[32 * i + 7 * bp:32 * i + 7 * bp + 7, c0 + 2 * bp + 1] = -wd
    for j in range(4):
        for bp in range(4):
            bias_all[32 * j + 7 * bp:32 * j + 7 * bp + 7, 0] = b1
            for l in range(N_MID):
                bias_all[32 * j + 7 * bp:32 * j + 7 * bp + 7, l + 1] = bmid[l]
            bias_all[32 * j + 2 * bp, 20] = delta
            bias_all[32 * j + 2 * bp + 1, 20] = -delta
    return w_all, bias_all


def _pack_x(x_core):
    """x_core [R_CORE, 2] -> xd [32, FREE]: partition 8i+2bp+u,
    free 2048s + 512j + jj, holding x[chunk(i,j,bp) row 512s+jj, u]."""
    # chunk q = 16i + 4j + bp ; rows 8192q + 512s + jj
    X = x_core.reshape(4, 4, 4, NSTR, SLOT, 2)       # [i, j, bp, s, jj, u]
    Xt = np.ascontiguousarray(np.transpose(X, (0, 2, 5, 3, 1, 4)))
    # [i, bp, u, s, j, jj] -> partitions (i, bp, u) = 8i+2bp+u, free (s, j, jj)
    return Xt.reshape(32, FREE)


def _unpack_out(od_list):
    """od [128, CHUNK] per core: row 32i+8j+2bp+sign, col 512s+jj =
    z=+/-d for chunk(i,j,bp) row 512s+jj. Host softplus. Return [B, 2]."""
    outs = []
    for od in od_list:
        A = od.reshape(4, 4, 4, 2, CHUNK)            # [i, j, bp, sign, col]
        At = np.transpose(A, (0, 1, 2, 4, 3))        # [i, j, bp, col, sign]
        outs.append(At.reshape(R_CORE, 2))
    z = np.concatenate(outs, axis=0)                 # z = +/-d (bias incl)
    return -np.logaddexp(np.float32(0), z)


def _build(reps=1):
    import concourse.bacc as bacc
    import concourse.tile as tile
    from concourse import mybir

    f32 = mybir.dt.float32
    AF = mybir.ActivationFunctionType
    ALU = mybir.AluOpType

    nc = bacc.Bacc("TRN2", target_bir_lowering=False, debug=False,
                   num_devices=N_CORES)
    xd = nc.dram_tensor("xd", [32, FREE], f32, kind="ExternalInput").ap()
    wd = nc.dram_tensor("wd", [128, 32 * N_LAYERS], f32,
                        kind="ExternalInput").ap()
    bd = nc.dram_tensor("bd", [128, N_LAYERS], f32, kind="ExternalInput").ap()
    od = nc.dram_tensor("od", [128, CHUNK], f32, kind="ExternalOutput").ap()

    with tile.TileContext(nc) as tc:
        with tc.tile_pool(name="wpool", bufs=1) as wpool, \
             tc.tile_pool(name="apool", bufs=1) as apool, \
             tc.tile_pool(name="xpool", bufs=6) as xpool, \
             tc.tile_pool(name="pspool", bufs=2, space="PSUM") as pspool:
            w_sb = wpool.tile([128, 32 * N_LAYERS], f32)
            nc.sync.dma_start(w_sb[:], wd[:])
            b_sb = wpool.tile([128, N_LAYERS], f32)
            nc.sync.dma_start(b_sb[:], bd[:])
            act = apool.tile([128, FREE], f32)

            mode = os.environ.get("KMODE", "full")
            pt_static = None
            mm_sink = None
            if mode == "decoupled":
                pt_static = [pspool.tile([128, SLOT], f32, name=f"pts{b}",
                                         tag=f"ptst{b}", bufs=1)
                             for b in range(4)]
                for b in range(4):
                    nc.vector.memset(pt_static[b][:, :], 0.0)
                mm_sink = pspool.tile([128, 4 * SLOT], f32, name="mmsink",
                                      tag="mmsink", bufs=1)
                nc.vector.memset(act[:, :], 0.0)
            if mode == "evac_only":
                pt_static = [pspool.tile([128, SLOT], f32, name=f"pts{b}",
                                         tag=f"pt{b}") for b in range(4)]
                for b in range(4):
                    nc.vector.memset(pt_static[b][:, :], 0.0)
            if mode == "mm_only":
                nc.vector.memset(act[:, :], 0.0)

            def body(_iv=None):
                for l in range(N_LAYERS):
                    K = 8 if l == 0 else 28
                    wcol = 32 * l
                    bias_ap = b_sb[:, l:l + 1]
                    for s in range(NSTR):
                        if l == 0:
                            xs = xpool.tile([128, 4 * SLOT], f32, name="xs",
                                            tag="xs")
                            for i in range(4):
                                eng = nc.sync if i % 2 == 0 else nc.gpsimd
                                eng.dma_start(
                                    xs[32 * i:32 * i + 8, :],
                                    xd[8 * i:8 * i + 8,
                                       4 * SLOT * s:4 * SLOT * (s + 1)])
                        if mode == "decoupled":
                            pt = pt_static
                        elif mode == "evac_only":
                            pt = pt_static
                        else:
                            pt = [pspool.tile([128, SLOT], f32,
                                              name=f"pt{b}", tag=f"pt{b}")
                                  for b in range(4)]
                        for r in range((0 if mode != "evac_only" else 4), 4):
                            for c in range(4):
                                mm_out = (mm_sink[32 * c:32 * c + 32,
                                                  SLOT * r:SLOT * (r + 1)]
                                          if mode == "decoupled" else
                                          pt[r][32 * c:32 * c + 32, :])
                                if l == 0:
                                    rhs = xs[32 * r:32 * r + 8,
                                             SLOT * c:SLOT * (c + 1)]
                                else:
                                    rhs = act[32 * r:32 * r + 28,
                                              4 * SLOT * s + SLOT * c:
                                              4 * SLOT * s + SLOT * (c + 1)]
                                nc.tensor.matmul(
                                    mm_out,
                                    lhsT=w_sb[32 * r:32 * r + K,
                                              wcol:wcol + 32],
                                    rhs=rhs, start=True, stop=True,
                                    tile_position=(32 * r, 32 * c))
                        if mode != "mm_only":
                            base = 4 * SLOT * s
                            for b in range(4):
                                dst = act[:, base + SLOT * b:
                                          base + SLOT * (b + 1)]
                                if l == N_LAYERS - 1:
                                    if b < 2:
                                        nc.scalar.activation(
                                            dst, pt[b][:, :], AF.Identity,
                                            bias=bias_ap, scale=1.0)
                                    else:
                                        nc.vector.tensor_scalar(
                                            out=dst, in0=pt[b][:, :],
                                            scalar1=bias_ap, scalar2=None,
                                            op0=ALU.add)
                                elif b < 2:
                                    nc.scalar.activation(
                                        dst, pt[b][:, :], AF.Relu,
                                        bias=bias_ap, scale=1.0)
                                else:
                                    nc.vector.tensor_scalar(
                                        out=dst, in0=pt[b][:, :],
                                        scalar1=bias_ap, scalar2=0.0,
                                        op0=ALU.add, op1=ALU.max)
                # output DMAs: group (i,j) -> od rows 32i+8j..+7
                for i in range(4):
                    for j in range(4):
                        src = act[32 * j:32 * j + 8, :].rearrange(
                            "p (s c jj) -> p s c jj", s=NSTR, c=4)[
                                :, :, i, :]
                        eng = nc.sync if (4 * i + j) % 2 == 0 else nc.gpsimd
                        eng.dma_start(
                            od[32 * i + 8 * j:32 * i + 8 * j + 8, :],
                            src)

            if reps == 1:
                body()
            else:
                with tc.For_i(0, reps, 1) as iv:
                    body(iv)
    nc.compile()
    return nc


def _make_runner(nc):
    """Reusable jitted 8-core shard_map runner (mirrors run_bass_via_pjrt)."""
    import jax
    import numpy as _np
    from jax.sharding import Mesh, PartitionSpec
    from jax.experimental.shard_map import shard_map
    import concourse.mybir as mybir
    from concourse.bass2jax import (_bass_exec_p, install_neuronx_cc_hook,
                                    partition_id_tensor)
    install_neuronx_cc_hook()

    partition_name = (nc.partition_id_tensor.name
                      if nc.partition_id_tensor else None)
    in_names, out_names, out_avals, zero_outs = [], [], [], []
    for alloc in nc.m.functions[0].allocations:
        if not isinstance(alloc, mybir.MemoryLocationSet):
            continue
        name = alloc.memorylocations[0].name
        if alloc.kind == "ExternalInput":
            if name != partition_name:
                in_names.append(name)
        elif alloc.kind == "ExternalOutput":
            shape = list(alloc.tensor_shape)
            np_dt = mybir.dt.np(alloc.dtype)
            out_avals.append(jax.core.ShapedArray(shape, np_dt))
            out_names.append(name)
            zero_outs.append(_np.zeros(shape, np_dt))
    all_in_names = (in_names + out_names +
                    ([partition_name] if partition_name else []))

    def _body(*args):
        operands = list(args)
        if partition_name is not None:
            operands.append(partition_id_tensor())
        return tuple(_bass_exec_p.bind(
            *operands, out_avals=tuple(out_avals),
            in_names=tuple(all_in_names), out_names=tuple(out_names),
            lowering_input_output_aliases=(),
            sim_require_finite=True, sim_require_nnan=True, nc=nc))

    devices = jax.devices()[:N_CORES]
    mesh = Mesh(np.asarray(devices), ("core",))
    n_in = len(in_names) + len(zero_outs)
    sharded = jax.jit(shard_map(
        _body, mesh=mesh, in_specs=(PartitionSpec("core"),) * n_in,
        out_specs=(PartitionSpec("core"),) * len(out_names),
        check_rep=False), keep_unused=True)

    def run(per_core_in_maps):
        concat_in = [np.concatenate([m[n] for m in per_core_in_maps], axis=0)
                     for n in in_names]
        concat_zero = [np.zeros((N_CORES * z.shape[0], *z.shape[1:]), z.dtype)
                       for z in zero_outs]
        outs = sharded(*concat_in, *concat_zero)
        res = []
        for c in range(N_CORES):
            res.append({n: np.asarray(outs[k]).reshape(
                N_CORES, *out_avals[k].shape)[c]
                for k, n in enumerate(out_names)})
        return res

    return run


def _get_runner(reps=1):
    key = ("runner", reps)
    if key not in _CACHE:
        nc = _build(reps)
        _CACHE[key] = _make_runner(nc)
    return _CACHE[key]


def kernel(x, W1, b1, Wmid, bmid, W21, b21):
    x = np.asarray(x, np.float32)
    w_all, bias_all = _pack_weights(
        np.asarray(W1, np.float32), np.asarray(b1, np.float32),
        np.asarray(Wmid, np.float32), np.asarray(bmid, np.float32),
        np.asarray(W21, np.float32), np.asarray(b21, np.float32))
    run = _get_runner(1)
    in_maps = []
    for core in range(N_CORES):
        xc = x[core * R_CORE:(core + 1) * R_CORE]
        in_maps.append({"xd": _pack_x(xc), "wd": w_all, "bd": bias_all})
    res = run(in_maps)
    return _unpack_out([res[c]["od"] for c in range(N_CORES)])


# revision 11
# speedup vs baseline: 1.2766x; 1.2766x over previous
"""Trainium2 Bass kernel for nn_Net0 (20-layer width-7 MLP + log_softmax).

Strategy: 8-way data parallel over batch. Per core, 64 batch chunks are
packed 4-chunks-per-32-partition-band (7 features each) so every 7x7 layer
becomes 16 concurrent 32x32 PE-tile matmuls (exact fp32, ~77ns per 512-col
MM measured). Activations stay resident in SBUF; bias+ReLU evacuation is
split across ScalarE and VectorE. Final log_softmax collapses to
-softplus(+/-d) with d = (W21[1]-W21[0]).h + (b21[1]-b21[0]); the sign flip
happens on the host.

Band ping-pong: PE tile (r,c) reads SBUF partitions 32r..32r+27 and writes
PSUM partition quadrant c (bank r). The partition-preserving evacuation of
bank b writes free-slot b, so a chunk processed by tile (r,c) at layer l is
processed by tile (c,r) at layer l+1. With tiles instantiated for all 16
(r,c) pairs each layer, the code is identical every layer.
"""

import os
import numpy as np

B = 4194304
H = 7
N_MID = 19
N_CORES = 8
R_CORE = B // N_CORES          # 524288
NSTR = 16                      # stripes per chunk
SLOT = 512                     # columns per stripe-slot
CHUNK = NSTR * SLOT            # 8192 rows per chunk; 64 chunks per core
FREE = 4 * CHUNK               # act free size (4 slots of CHUNK, stripe-major)
N_LAYERS = 21                  # fc1 + 19 mid + fin

_CACHE = {}


def _pack_weights(W1, b1, Wmid, bmid, W21, b21):
    """Build w_all [128, 588] and bias_all [128, 21] host-side."""
    w_all = np.zeros((128, 32 * N_LAYERS), np.float32)
    bias_all = np.zeros((128, N_LAYERS), np.float32)
    wd = (W21[1] - W21[0]).astype(np.float32)        # [7]
    delta = np.float32(b21[1] - b21[0])
    for i in range(4):                                # band
        for bp in range(4):                           # block within band
            # fc1: K rows 32i+2bp+u, M cols 7bp+f  -> W1[f, u]
            for u in range(2):
                w_all[32 * i + 2 * bp + u, 7 * bp:7 * bp + 7] = W1[:, u]
            for l in range(N_MID):
                c0 = 32 * (l + 1)
                # rows 32i+7bp+fin, cols c0+7bp+fout -> Wmid[l, fout, fin]
                w_all[32 * i + 7 * bp:32 * i + 7 * bp + 7,
                      c0 + 7 * bp:c0 + 7 * bp + 7] = Wmid[l].T
            # fin: rows 32i+7bp+f, cols 560 + 2bp+sign -> +/- wd[f]
            c0 = 32 * 20
            w_all[32 * i + 7 * bp:32 * i + 7 * bp + 7, c0 + 2 * bp] = wd
            w_all[32 * i + 7 * bp:32 * i + 7 * bp + 7, c0 + 2 * bp + 1] = -wd
    for j in range(4):
        for bp in range(4):
            bias_all[32 * j + 7 * bp:32 * j + 7 * bp + 7, 0] = b1
            for l in range(N_MID):
                bias_all[32 * j + 7 * bp:32 * j + 7 * bp + 7, l + 1] = bmid[l]
            bias_all[32 * j + 2 * bp, 20] = delta
            bias_all[32 * j + 2 * bp + 1, 20] = -delta
    return w_all, bias_all


def _pack_x(x_core):
    """x_core [R_CORE, 2] -> xd [32, FREE]: partition 8i+2bp+u,
    free 2048s + 512j + jj, holding x[chunk(i,j,bp) row 512s+jj, u]."""
    # chunk q = 16i + 4j + bp ; rows 8192q + 512s + jj
    X = x_core.reshape(4, 4, 4, NSTR, SLOT, 2)       # [i, j, bp, s, jj, u]
    Xt = np.ascontiguousarray(np.transpose(X, (0, 2, 5, 3, 1, 4)))
    # [i, bp, u, s, j, jj] -> partitions (i, bp, u) = 8i+2bp+u, free (s, j, jj)
    return Xt.reshape(32, FREE)


def _unpack_out(od_list):
    """od [128, CHUNK] per core: row 32i+8j+2bp+sign, col 512s+jj =
    z=+/-d for chunk(i,j,bp) row 512s+jj. Host softplus. Return [B, 2]."""
    outs = []
    for od in od_list:
        A = od.reshape(4, 4, 4, 2, CHUNK)            # [i, j, bp, sign, col]
        At = np.transpose(A, (0, 1, 2, 4, 3))        # [i, j, bp, col, sign]
        outs.append(At.reshape(R_CORE, 2))
    z = np.concatenate(outs, axis=0)                 # z = +/-d (bias incl)
    return -np.logaddexp(np.float32(0), z)


def _build(reps=1):
    import concourse.bacc as bacc
    import concourse.tile as tile
    from concourse import mybir

    f32 = mybir.dt.float32
    AF = mybir.ActivationFunctionType
    ALU = mybir.AluOpType

    nc = bacc.Bacc("TRN2", target_bir_lowering=False, debug=False,
                   num_devices=N_CORES)
    xd = nc.dram_tensor("xd", [32, FREE], f32, kind="ExternalInput").ap()
    wd = nc.dram_tensor("wd", [128, 32 * N_LAYERS], f32,
                        kind="ExternalInput").ap()
    bd = nc.dram_tensor("bd", [128, N_LAYERS], f32, kind="ExternalInput").ap()
    od = nc.dram_tensor("od", [128, CHUNK], f32, kind="ExternalOutput").ap()

    with tile.TileContext(nc) as tc:
        with tc.tile_pool(name="wpool", bufs=1) as wpool, \
             tc.tile_pool(name="apool", bufs=1) as apool, \
             tc.tile_pool(name="xpool", bufs=6) as xpool, \
             tc.tile_pool(name="pspool", bufs=2, space="PSUM") as pspool:
            w_sb = wpool.tile([128, 32 * N_LAYERS], f32)
            nc.sync.dma_start(w_sb[:], wd[:])
            b_sb = wpool.tile([128, N_LAYERS], f32)
            nc.sync.dma_start(b_sb[:], bd[:])
            act = apool.tile([128, FREE], f32)

            mode = os.environ.get("KMODE", "full")
            pt_static = None
            mm_sink = None
            if mode in ("decoupled", "dec_sbufsrc"):
                if mode == "decoupled":
                    pt_static = [pspool.tile([128, SLOT], f32,
                                             name=f"pts{b}",
                                             tag=f"ptst{b}", bufs=1)
                                 for b in range(4)]
                else:
                    pt_static = [wpool.tile([128, SLOT], f32,
                                            name=f"ptsb{b}")
                                 for b in range(4)]
                for b in range(4):
                    nc.vector.memset(pt_static[b][:, :], 0.0)
                mm_sink = pspool.tile([128, 4 * SLOT], f32, name="mmsink",
                                      tag="mmsink", bufs=1)
                nc.vector.memset(act[:, :], 0.0)
            if mode == "evac_only":
                pt_static = [pspool.tile([128, SLOT], f32, name=f"pts{b}",
                                         tag=f"pt{b}") for b in range(4)]
                for b in range(4):
                    nc.vector.memset(pt_static[b][:, :], 0.0)
            if mode == "mm_only":
                nc.vector.memset(act[:, :], 0.0)

            def body(_iv=None):
                for l in range(N_LAYERS):
                    K = 8 if l == 0 else 28
                    wcol = 32 * l
                    bias_ap = b_sb[:, l:l + 1]
                    for s in range(NSTR):
                        if l == 0:
                            xs = xpool.tile([128, 4 * SLOT], f32, name="xs",
                                            tag="xs")
                            for i in range(4):
                                eng = nc.sync if i % 2 == 0 else nc.gpsimd
                                eng.dma_start(
                                    xs[32 * i:32 * i + 8, :],
                                    xd[8 * i:8 * i + 8,
                                       4 * SLOT * s:4 * SLOT * (s + 1)])
                        if mode in ("decoupled", "dec_sbufsrc"):
                            pt = pt_static
                        elif mode == "evac_only":
                            pt = pt_static
                        else:
                            pt = [pspool.tile([128, SLOT], f32,
                                              name=f"pt{b}", tag=f"pt{b}")
                                  for b in range(4)]
                        for r in range((0 if mode != "evac_only" else 4), 4):
                            for c in range(4):
                                mm_out = (mm_sink[32 * c:32 * c + 32,
                                                  SLOT * r:SLOT * (r + 1)]
                                          if mode in ("decoupled",
                                                      "dec_sbufsrc") else
                                          pt[r][32 * c:32 * c + 32, :])
                                if l == 0:
                                    rhs = xs[32 * r:32 * r + 8,
                                             SLOT * c:SLOT * (c + 1)]
                                else:
                                    rhs = act[32 * r:32 * r + 28,
                                              4 * SLOT * s + SLOT * c:
                                              4 * SLOT * s + SLOT * (c + 1)]
                                nc.tensor.matmul(
                                    mm_out,
                                    lhsT=w_sb[32 * r:32 * r + K,
                                              wcol:wcol + 32],
                                    rhs=rhs, start=True, stop=True,
                                    tile_position=(32 * r, 32 * c))
                        if mode != "mm_only":
                            base = 4 * SLOT * s
                            for b in range(4):
                                dst = act[:, base + SLOT * b:
                                          base + SLOT * (b + 1)]
                                if l == N_LAYERS - 1:
                                    if b < 2:
                                        nc.scalar.activation(
                                            dst, pt[b][:, :], AF.Identity,
                                            bias=bias_ap, scale=1.0)
                                    else:
                                        nc.vector.tensor_scalar(
                                            out=dst, in0=pt[b][:, :],
                                            scalar1=bias_ap, scalar2=None,
                                            op0=ALU.add)
                                elif b < 2:
                                    nc.scalar.activation(
                                        dst, pt[b][:, :], AF.Relu,
                                        bias=bias_ap, scale=1.0)
                                else:
                                    nc.vector.tensor_scalar(
                                        out=dst, in0=pt[b][:, :],
                                        scalar1=bias_ap, scalar2=0.0,
                                        op0=ALU.add, op1=ALU.max)
                # output DMAs: group (i,j) -> od rows 32i+8j..+7
                for i in range(4):
                    for j in range(4):
                        src = act[32 * j:32 * j + 8, :].rearrange(
                            "p (s c jj) -> p s c jj", s=NSTR, c=4)[
                                :, :, i, :]
                        eng = nc.sync if (4 * i + j) % 2 == 0 else nc.gpsimd
                        eng.dma_start(
                            od[32 * i + 8 * j:32 * i + 8 * j + 8, :],
                            src)

            if reps == 1:
                body()
            else:
                with tc.For_i(0, reps, 1) as iv:
                    body(iv)
    nc.compile()
    return nc


def _make_runner(nc):
    """Reusable jitted 8-core shard_map runner (mirrors run_bass_via_pjrt)."""
    import jax
    import numpy as _np
    from jax.sharding import Mesh, PartitionSpec
    from jax.experimental.shard_map import shard_map
    import concourse.mybir as mybir
    from concourse.bass2jax import (_bass_exec_p, install_neuronx_cc_hook,
                                    partition_id_tensor)
    install_neuronx_cc_hook()

    partition_name = (nc.partition_id_tensor.name
                      if nc.partition_id_tensor else None)
    in_names, out_names, out_avals, zero_outs = [], [], [], []
    for alloc in nc.m.functions[0].allocations:
        if not isinstance(alloc, mybir.MemoryLocationSet):
            continue
        name = alloc.memorylocations[0].name
        if alloc.kind == "ExternalInput":
            if name != partition_name:
                in_names.append(name)
        elif alloc.kind == "ExternalOutput":
            shape = list(alloc.tensor_shape)
            np_dt = mybir.dt.np(alloc.dtype)
            out_avals.append(jax.core.ShapedArray(shape, np_dt))
            out_names.append(name)
            zero_outs.append(_np.zeros(shape, np_dt))
    all_in_names = (in_names + out_names +
                    ([partition_name] if partition_name else []))

    def _body(*args):
        operands = list(args)
        if partition_name is not None:
            operands.append(partition_id_tensor())
        return tuple(_bass_exec_p.bind(
            *operands, out_avals=tuple(out_avals),
            in_names=tuple(all_in_names), out_names=tuple(out_names),
            lowering_input_output_aliases=(),
            sim_require_finite=True, sim_require_nnan=True, nc=nc))

    devices = jax.devices()[:N_CORES]
    mesh = Mesh(np.asarray(devices), ("core",))
    n_in = len(in_names) + len(zero_outs)
    sharded = jax.jit(shard_map(
        _body, mesh=mesh, in_specs=(PartitionSpec("core"),) * n_in,
        out_specs=(PartitionSpec("core"),) * len(out_names),
        check_rep=False), keep_unused=True)

    def run(per_core_in_maps):
        concat_in = [np.concatenate([m[n] for m in per_core_in_maps], axis=0)
                     for n in in_names]
        concat_zero = [np.zeros((N_CORES * z.shape[0], *z.shape[1:]), z.dtype)
                       for z in zero_outs]
        outs = sharded(*concat_in, *concat_zero)
        res = []
        for c in range(N_CORES):
            res.append({n: np.asarray(outs[k]).reshape(
                N_CORES, *out_avals[k].shape)[c]
                for k, n in enumerate(out_names)})
        return res

    return run


def _get_runner(reps=1):
    key = ("runner", reps)
    if key not in _CACHE:
        nc = _build(reps)
        _CACHE[key] = _make_runner(nc)
    return _CACHE[key]


def kernel(x, W1, b1, Wmid, bmid, W21, b21):
    x = np.asarray(x, np.float32)
    w_all, bias_all = _pack_weights(
        np.asarray(W1, np.float32), np.asarray(b1, np.float32),
        np.asarray(Wmid, np.float32), np.asarray(bmid, np.float32),
        np.asarray(W21, np.float32), np.asarray(b21, np.float32))
    run = _get_runner(1)
    in_maps = []
    for core in range(N_CORES):
        xc = x[core * R_CORE:(core + 1) * R_CORE]
        in_maps.append({"xd": _pack_x(xc), "wd": w_all, "bd": bias_all})
    res = run(in_maps)
    return _unpack_out([res[c]["od"] for c in range(N_CORES)])
